# revision 23
# baseline (speedup 1.0000x reference)
"""Causal self-attention kernel for Trainium2, distributed over 8 NeuronCores.

Problem (full): x[2, 2048, 1024], Wq/Wk/Wv[1024, 16, 64], Wo[16, 64, 1024]
  q/k/v = einsum('bld,dhk->blhk'); scores = q k^T / sqrt(64), causal mask,
  softmax; y = attn @ v; out = einsum('blhk,hkd->bld').

Sharding: core c in 0..7 -> batch b = c // 4, head-group g = c % 4
  (heads [4g, 4g+4)).  Each core computes its batch's partial output
  projection over its 4 heads; the host sums the 4 head-group partials
  per batch (the "all-reduce" of the output projection done host-side
  during unsharding).

Per-core design (bf16 matmuls, f32 PSUM accumulation):
  - All dram tensors are PRE-LAID-OUT host-side in on-chip order so every
    DMA is contiguous per partition (large packets), and the load order
    interleaves weights with x slices so the first projection matmul is
    gated on ~1.5MB, not ~5.5MB.
  - Q^T, K^T computed as [128(d of head-pair), 2, 2048]; scores for the
    two heads of a pair are issued to disjoint PE row groups (K=64,
    base partitions 0/64) so they run CONCURRENTLY.
  - softmax without max-subtraction (scores are O(10): exp is safe).
    The exp work is split across BOTH the Scalar engine (table exp) and
    the Vector engine (exp2 bit-trick: y = round(s*128*log2e + magic) as
    int16, bitcast to bf16 -- |rel err| <= 3.3%, washes out in softmax
    normalization + AV averaging), greedily balanced at build time.
  - causal mask: no additive -inf pass; instead gpsimd affine_select
    zeroes the upper triangle of the exp'd diagonal strip in SBUF
    (gpsimd is otherwise idle; DVE/ACT keep their cycles for exp).
  - denominator free via ones-column appended to V: AV matmul produces
    [y(64 rows); rowsum(1 row)] per head into one [65,2,512] PSUM tile.
  - normalization: psys is copied to SBUF immediately (frees PSUM for the
    next (a,t) AV), then reciprocal (DVE) + partition_broadcast (gpsimd)
    + multiply (DVE) produce normalized Y^T in bf16 -- exactly the lhsT
    layout the output projection needs.
  - emission order is software-pipelined: outproj(a) is emitted after
    attention(a+1, t=0), so the PE always has queued matmuls while the
    normalization chain of block a completes (keeps HAM un-throttled).
"""

import sys

sys.path.insert(0, "/opt/trn_rl_repo")

import os

import ml_dtypes
import numpy as np
from contextlib import ExitStack

_NO_DVE_EXP = os.environ.get("K_NO_DVE_EXP", "0") == "1"
_NO_TRI = os.environ.get("K_NO_TRI", "0") == "1"

import concourse.bass as bass
import concourse.mybir as mybir
import concourse.tile as tile
from concourse import bacc

F32 = mybir.dt.float32
BF16 = mybir.dt.bfloat16
I16 = mybir.dt.int16
AF = mybir.ActivationFunctionType

B, L, D, H, HD = 2, 2048, 1024, 16, 64
NCORES = 8
HG = 4              # heads per core
NG = H // HG        # 4 head-groups
T = HG // 2         # 2 head-pairs per core
P = 128
KC = D // P         # 8 contraction chunks for the projections
QB = 512            # query-range block (moving free dim)
NA = L // QB        # 4 query ranges
NJ = L // P         # 16 key blocks
SCALE = 1.0 / np.sqrt(HD)

# exp2 bit-trick constants (bf16 exponent starts at bit 7)
LOG2E = float(np.log2(np.e))
EXP_K = float(128.0 * LOG2E)          # multiplied by SCALE at the call site
EXP_B = float(127 * 128 - 5.1)        # minimax offset for truncation


class _ExpBalancer:
    """Greedy build-time assignment of exp blocks to ACT vs DVE."""

    def __init__(self):
        self.act_ns = 0.0
        self.dve_ns = 0.0

    def pick(self, elems):
        act_cost = (elems + 352) / 1.2
        dve_cost = (elems + 271) / 0.96
        if self.act_ns + act_cost <= self.dve_ns + dve_cost:
            self.act_ns += act_cost
            return "act"
        self.dve_ns += dve_cost
        return "dve"

    def charge(self, act_elems, dve_elems):
        self.act_ns += (act_elems + 352) / 1.2
        self.dve_ns += (dve_elems + 271) / 0.96


def _body(ctx: ExitStack, tc: tile.TileContext, xt_d, wq_d, wk_d, wv_d, wo_d, out_d):
    nc = tc.nc

    consts = ctx.enter_context(tc.tile_pool(name="consts", bufs=1))
    pj = ctx.enter_context(tc.tile_pool(name="pj", bufs=2, space="PSUM"))
    ps = ctx.enter_context(tc.tile_pool(name="ps", bufs=2, space="PSUM"))
    py = ctx.enter_context(tc.tile_pool(name="py", bufs=1, space="PSUM"))
    ptp = ctx.enter_context(tc.tile_pool(name="ptp", bufs=7))
    smp = ctx.enter_context(tc.tile_pool(name="smp", bufs=4))
    obp = ctx.enter_context(tc.tile_pool(name="obp", bufs=3))

    # ---- resident inputs; DMA order gates the first matmuls on ~1.5MB
    wq = consts.tile([P, T, KC, P], BF16)
    wk = consts.tile([P, T, KC, P], BF16)
    wv = consts.tile([P, KC, HG * HD], BF16)
    wo = consts.tile([P, T, D], BF16)
    xt = consts.tile([P, NA, KC, QB], BF16)   # x^T: [p, m, c, l-within-m]
    # the first matmul only needs wk(t=0) + x(m=0): ~1.25MB gates the PE
    nc.sync.dma_start(out=wk[:, 0], in_=wk_d[0])
    nc.sync.dma_start(out=xt[:, 0], in_=xt_d[:, 0])
    nc.sync.dma_start(out=wq[:, 0], in_=wq_d[0])
    nc.sync.dma_start(out=xt[:, 1], in_=xt_d[:, 1])
    nc.sync.dma_start(out=xt[:, 2], in_=xt_d[:, 2])
    nc.sync.dma_start(out=wk[:, 1], in_=wk_d[1])
    nc.sync.dma_start(out=wq[:, 1], in_=wq_d[1])
    nc.sync.dma_start(out=xt[:, 3], in_=xt_d[:, 3])
    nc.sync.dma_start(out=wv, in_=wv_d)
    nc.sync.dma_start(out=wo, in_=wo_d)

    # ---- intermediates
    qt = consts.tile([P, T, L], BF16)         # Q^T: [d-of-pair, t, m]
    kt = consts.tile([P, T, L], BF16)
    vsb = consts.tile([P, NJ, HG, HD + 1], BF16)  # [key-in-blk, jb, h, d | ones]
    yt = consts.tile([P, T, L], BF16)         # normalized Y^T
    nc.vector.memset(vsb[:, :, :, HD:HD + 1], 1.0)

    cp_rot = [0]

    def copy_out(dst, src):
        # alternate PSUM->SBUF copies between ACT and DVE
        if cp_rot[0] % 2 == 0:
            nc.scalar.copy(out=dst, in_=src)
        else:
            nc.vector.tensor_copy(out=dst, in_=src)
        cp_rot[0] += 1

    # ---- projections (bf16, contraction over D in 8 chunks of 128)
    for t in range(T):
        for m in range(NA):
            msl = slice(m * QB, (m + 1) * QB)
            pk = pj.tile([P, QB], F32, tag="pj")
            for c in range(KC):
                nc.tensor.matmul(pk, lhsT=wk[:, t, c, :],
                                 rhs=xt[:, m, c, :], start=(c == 0), stop=(c == KC - 1))
            copy_out(kt[:, t, msl], pk)
            pq = pj.tile([P, QB], F32, tag="pj")
            for c in range(KC):
                nc.tensor.matmul(pq, lhsT=wq[:, t, c, :],
                                 rhs=xt[:, m, c, :], start=(c == 0), stop=(c == KC - 1))
            copy_out(qt[:, t, msl], pq)
    for jb in range(NJ):
        m, q = jb // 4, jb % 4
        pv = pj.tile([P, HG * HD], F32, tag="pj")
        for c in range(KC):
            nc.tensor.matmul(pv, lhsT=xt[:, m, c, q * P:(q + 1) * P],
                             rhs=wv[:, c, :], start=(c == 0), stop=(c == KC - 1))
        copy_out(vsb[:, jb, :, 0:HD], pv.rearrange("p (h d) -> p h d", h=HG))

    bal = _ExpBalancer()
    bal.dve_ns = 2500.0   # early norm chains ride on DVE; bias first exps to ACT
    pending = []   # deferred normalization state: dict(a, t, drow, ysb, den)

    def norm_recip_bcast(pn):
        # step A (emitted at the START of the next attention block, so the
        # gpsimd broadcast clears the queue before that block's zeros):
        rec = smp.tile([1, 2, QB], F32, tag="rec")
        nc.vector.reciprocal_approx_fast(
            out=rec.rearrange("p a b -> p (a b)"),
            in_=pn["drow"].rearrange("p a b -> p (a b)"))
        den = smp.tile([64, 2, QB], F32, tag="den")
        nc.gpsimd.partition_broadcast(den, rec)
        pn["den"] = den

    def norm_muls(pn):
        # step B (emitted at the END of the next attention block: by then
        # the broadcast is long done, so these never block the DVE queue)
        a, t = pn["a"], pn["t"]
        for u in range(2):
            # all-SBUF multiply: run it on gpsimd so the DVE keeps its
            # cycles for exp (DVE is the scarce engine in short blocks)
            nc.gpsimd.tensor_mul(yt[64 * u:64 * u + 64, t, a * QB:(a + 1) * QB],
                                 pn["ysb"][:, u, :], pn["den"][:, u, :])

    LOOK = 5   # AV(j) is emitted after scores(j+LOOK): the PE (strict FIFO)
               # executes 3 score-pairs' worth of work while exp(j) completes,
               # so AV(j) never stalls the PE on the exp latency.

    def attn(a, t, last):
        nj = 4 * a + 4
        if pending:
            norm_recip_bcast(pending[-1])
        psys = py.tile([65, 2, QB], F32, tag="py", name=f"psy{a}{t}")

        def emit_av(j, pt, off):
            for u in range(2):
                nc.tensor.matmul(
                    psys[:, u, off:QB],
                    lhsT=vsb[:, j, 2 * t + u, :],
                    rhs=pt[:, u, off:QB],
                    start=(j == 0), stop=(j == nj - 1),
                )

        def tail_chain(mi):
            # last block only: cols [128*mi, 128*(mi+1)) of psys are final
            # once AV(j=nj-4+mi) has run, so normalize them m-granularly
            # while the remaining AVs still stream on the PE
            msl = slice(mi * P, (mi + 1) * P)
            drow = smp.tile([1, 2, P], F32, tag="drowm")
            nc.vector.tensor_copy(out=drow, in_=psys[64:65, :, msl])
            rec = smp.tile([1, 2, P], F32, tag="recm")
            nc.vector.reciprocal_approx_fast(
                out=rec.rearrange("p a b -> p (a b)"),
                in_=drow.rearrange("p a b -> p (a b)"))
            den = smp.tile([64, 2, P], F32, tag="denm")
            nc.gpsimd.partition_broadcast(den, rec)
            for u in range(2):
                nc.vector.tensor_mul(
                    yt[64 * u:64 * u + 64, t,
                       a * QB + mi * P:a * QB + (mi + 1) * P],
                    psys[0:64, u, msl], den[:, u, :])

        stash = []
        for j in range(nj):
            r = j - 4 * a          # >= 0 on diagonal blocks
            off = 128 * r if r >= 0 else 0
            pss = ps.tile([P, 2, QB], F32, tag="ps")
            for u in range(2):
                hp = slice(64 * u, 64 * u + 64)
                nc.tensor.matmul(
                    pss[:, u, off:QB],
                    lhsT=kt[hp, t, j * P:(j + 1) * P],
                    rhs=qt[hp, t, a * QB + off:(a + 1) * QB],
                    start=True, stop=True,
                )
            pt = ptp.tile([P, 2, QB], BF16, tag="pt")
            elems = 2 * (QB - off)
            if bal.pick(elems) == "act":
                nc.scalar.activation(pt[:, :, off:QB], pss[:, :, off:QB],
                                     AF.Exp, scale=float(SCALE))
            else:
                nc.vector.tensor_scalar(
                    out=pt[:, :, off:QB].bitcast(I16), in0=pss[:, :, off:QB],
                    scalar1=float(SCALE * EXP_K), scalar2=EXP_B,
                    op0=mybir.AluOpType.mult, op1=mybir.AluOpType.add)
            if r >= 0:
                # zero the causal upper triangle of the diagonal strip:
                # keep iff (query-within-strip - key) >= 0
                nc.gpsimd.affine_select(
                    out=pt[:, :, off:off + P], in_=pt[:, :, off:off + P],
                    compare_op=mybir.AluOpType.is_ge, fill=0.0,
                    base=0, pattern=[[0, 2], [1, P]], channel_multiplier=-1)
            stash.append((j, pt, off))
            if len(stash) > LOOK:
                emit_av(*stash.pop(0))
        if last:
            if pending:
                norm_muls(pending.pop(0))
            for s in stash:
                emit_av(*s)
            for mi in range(4):
                tail_chain(mi)
                outproj_m(4 * a + mi)
            return
        for s in stash:
            emit_av(*s)
        if not last:
            # free psys ASAP: its only readers are these two copies
            ysb = smp.tile([64, 2, QB], F32, tag="ysb")
            nc.scalar.copy(out=ysb, in_=psys[0:64])
            drow = smp.tile([1, 2, QB], F32, tag="drow")
            nc.vector.tensor_copy(out=drow.rearrange("p a b -> p (a b)"),
                                  in_=psys[64:65].rearrange("p a b -> p (a b)"))
            if pending:
                norm_muls(pending.pop(0))
            pending.append({"a": a, "t": t, "drow": drow, "ysb": ysb})

    def outproj_m(m):
        for db in range(2):
            dsl = slice(db * QB, (db + 1) * QB)
            pso = pj.tile([P, QB], F32, tag="pj")
            for t in range(T):
                nc.tensor.matmul(
                    pso,
                    lhsT=yt[:, t, m * P:(m + 1) * P],
                    rhs=wo[:, t, dsl],
                    start=(t == 0), stop=(t == T - 1),
                )
            ob = obp.tile([P, QB], BF16, tag="ob")
            copy_out(ob, pso)
            nc.sync.dma_start(out=out_d[m, :, db], in_=ob)

    def outproj(a):
        for mi in range(4):
            outproj_m(4 * a + mi)

    # software-pipelined emission: outproj(a) after attn(a+1, t=0)
    attn(0, 0, False)
    attn(0, 1, False)
    attn(1, 0, False)
    outproj(0)
    attn(1, 1, False)
    attn(2, 0, False)
    outproj(1)
    attn(2, 1, False)
    attn(3, 0, False)
    outproj(2)
    attn(3, 1, True)


_NC_CACHE = None


def _build_nc():
    global _NC_CACHE
    if _NC_CACHE is not None:
        return _NC_CACHE
    nc = bacc.Bacc("TRN2", target_bir_lowering=False, debug=False,
                   enable_asserts=False)
    xt_d = nc.dram_tensor("xt", [P, NA, KC, QB], BF16, kind="ExternalInput")
    wq_d = nc.dram_tensor("wq", [T, P, KC, P], BF16, kind="ExternalInput")
    wk_d = nc.dram_tensor("wk", [T, P, KC, P], BF16, kind="ExternalInput")
    wv_d = nc.dram_tensor("wv", [P, KC, HG * HD], BF16, kind="ExternalInput")
    wo_d = nc.dram_tensor("wo", [P, T, D], BF16, kind="ExternalInput")
    out_d = nc.dram_tensor("out", [NJ, P, 2, QB], BF16, kind="ExternalOutput")
    with tile.TileContext(nc) as tc, ExitStack() as ctx:
        _body(ctx, tc, xt_d.ap(), wq_d.ap(), wk_d.ap(), wv_d.ap(), wo_d.ap(),
              out_d.ap())
    nc.compile()
    _NC_CACHE = nc
    return nc


def _shard_inputs(x_bld, Wq, Wk, Wv, Wo):
    x_bld = np.asarray(x_bld, dtype=np.float32)
    Wq = np.asarray(Wq, dtype=np.float32)
    Wk = np.asarray(Wk, dtype=np.float32)
    Wv = np.asarray(Wv, dtype=np.float32)
    Wo = np.asarray(Wo, dtype=np.float32)
    bf = ml_dtypes.bfloat16

    def wlay(Wm):       # [1024, 256] -> [128, 8, 256] (p, c, n)
        return np.ascontiguousarray(
            Wm.reshape(KC, P, HG * HD).transpose(1, 0, 2).astype(bf))

    def wlay_t(Wm):     # [1024, 256] -> [2, 128, 8, 128] (t, p, c, n)
        return np.ascontiguousarray(
            Wm.reshape(KC, P, T, P).transpose(2, 1, 0, 3).astype(bf))

    in_maps = []
    for c in range(NCORES):
        b, g = divmod(c, NG)
        hsl = slice(g * HG, (g + 1) * HG)
        # x^T [1024, 2048] -> [128, 4, 8, 512] (p, m, c, l)
        xT = x_bld[b].T.reshape(KC, P, NA, QB)
        xl = np.ascontiguousarray(xT.transpose(1, 2, 0, 3).astype(bf))
        # Wo [4, 64, 1024] -> [256, 1024] -> [128, 2, 1024] (p, t, d)
        woR = Wo[hsl].reshape(T, P, D)
        wol = np.ascontiguousarray(woR.transpose(1, 0, 2).astype(bf))
        in_maps.append({
            "xt": xl,
            "wq": wlay_t(Wq[:, hsl, :].reshape(D, HG * HD)),
            "wk": wlay_t(Wk[:, hsl, :].reshape(D, HG * HD)),
            "wv": wlay(Wv[:, hsl, :].reshape(D, HG * HD)),
            "wo": wol,
        })
    return in_maps


def _combine(outs):
    y = np.zeros((B, L, D), dtype=np.float32)
    for c in range(NCORES):
        y[c // NG] += np.asarray(outs[c], dtype=np.float32).reshape(L, D)
    return y


LAST_RESULT = None


def kernel(x_bld, Wq, Wk, Wv, Wo):
    global LAST_RESULT
    from concourse.bass_utils import run_bass_kernel_spmd
    nc = _build_nc()
    in_maps = _shard_inputs(x_bld, Wq, Wk, Wv, Wo)
    res = run_bass_kernel_spmd(nc, in_maps, core_ids=list(range(NCORES)))
    LAST_RESULT = res
    return _combine([res.results[c]["out"] for c in range(NCORES)])


# revision 24
# speedup vs baseline: 1.4361x; 1.4361x over previous
"""Causal self-attention kernel for Trainium2, distributed over 8 NeuronCores.

Problem (full): x[2, 2048, 1024], Wq/Wk/Wv[1024, 16, 64], Wo[16, 64, 1024]
  q/k/v = einsum('bld,dhk->blhk'); scores = q k^T / sqrt(64), causal mask,
  softmax; y = attn @ v; out = einsum('blhk,hkd->bld').

Sharding: core c in 0..7 -> batch b = c // 4, head-group g = c % 4
  (heads [4g, 4g+4)).  Each core computes its batch's partial output
  projection over its 4 heads; the host sums the 4 head-group partials
  per batch (the "all-reduce" of the output projection done host-side
  during unsharding).

Per-core design (bf16 matmuls, f32 PSUM accumulation):
  - All dram tensors are PRE-LAID-OUT host-side in on-chip order so every
    DMA is contiguous per partition (large packets), and the load order
    interleaves weights with x slices so the first projection matmul is
    gated on ~1.5MB, not ~5.5MB.
  - Q^T, K^T computed as [128(d of head-pair), 2, 2048]; scores for the
    two heads of a pair are issued to disjoint PE row groups (K=64,
    base partitions 0/64) so they run CONCURRENTLY.
  - softmax without max-subtraction (scores are O(10): exp is safe).
    The exp work is split across BOTH the Scalar engine (table exp) and
    the Vector engine (exp2 bit-trick: y = round(s*128*log2e + magic) as
    int16, bitcast to bf16 -- |rel err| <= 3.3%, washes out in softmax
    normalization + AV averaging), greedily balanced at build time.
  - causal mask: no additive -inf pass; instead gpsimd affine_select
    zeroes the upper triangle of the exp'd diagonal strip in SBUF
    (gpsimd is otherwise idle; DVE/ACT keep their cycles for exp).
  - denominator free via ones-column appended to V: AV matmul produces
    [y(64 rows); rowsum(1 row)] per head into one [65,2,512] PSUM tile.
  - normalization: psys is copied to SBUF immediately (frees PSUM for the
    next (a,t) AV), then reciprocal (DVE) + partition_broadcast (gpsimd)
    + multiply (DVE) produce normalized Y^T in bf16 -- exactly the lhsT
    layout the output projection needs.
  - emission order is software-pipelined: outproj(a) is emitted after
    attention(a+1, t=0), so the PE always has queued matmuls while the
    normalization chain of block a completes (keeps HAM un-throttled).
"""

import sys

sys.path.insert(0, "/opt/trn_rl_repo")

import os

import ml_dtypes
import numpy as np
from contextlib import ExitStack

_NO_DVE_EXP = os.environ.get("K_NO_DVE_EXP", "0") == "1"
_NO_TRI = os.environ.get("K_NO_TRI", "0") == "1"

import concourse.bass as bass
import concourse.mybir as mybir
import concourse.tile as tile
from concourse import bacc

F32 = mybir.dt.float32
BF16 = mybir.dt.bfloat16
I16 = mybir.dt.int16
AF = mybir.ActivationFunctionType

B, L, D, H, HD = 2, 2048, 1024, 16, 64
NCORES = 8
HG = 4              # heads per core
NG = H // HG        # 4 head-groups
T = HG // 2         # 2 head-pairs per core
P = 128
KC = D // P         # 8 contraction chunks for the projections
QB = 512            # query-range block (moving free dim)
NA = L // QB        # 4 query ranges
NJ = L // P         # 16 key blocks
SCALE = 1.0 / np.sqrt(HD)

# exp2 bit-trick constants (bf16 exponent starts at bit 7)
LOG2E = float(np.log2(np.e))
EXP_K = float(128.0 * LOG2E)          # multiplied by SCALE at the call site
EXP_B = float(127 * 128 - 5.1)        # minimax offset for truncation


class _ExpBalancer:
    """Greedy build-time assignment of exp blocks to ACT vs DVE."""

    def __init__(self):
        self.act_ns = 0.0
        self.dve_ns = 0.0

    def pick(self, elems):
        act_cost = (elems + 352) / 1.2
        dve_cost = (elems + 271) / 0.96
        if self.act_ns + act_cost <= self.dve_ns + dve_cost:
            self.act_ns += act_cost
            return "act"
        self.dve_ns += dve_cost
        return "dve"

    def charge(self, act_elems, dve_elems):
        self.act_ns += (act_elems + 352) / 1.2
        self.dve_ns += (dve_elems + 271) / 0.96


def _body(ctx: ExitStack, tc: tile.TileContext, xt_d, wq_d, wk_d, wv_d, wo_d, out_d):
    nc = tc.nc

    consts = ctx.enter_context(tc.tile_pool(name="consts", bufs=1))
    pj = ctx.enter_context(tc.tile_pool(name="pj", bufs=2, space="PSUM"))
    ps = ctx.enter_context(tc.tile_pool(name="ps", bufs=2, space="PSUM"))
    py = ctx.enter_context(tc.tile_pool(name="py", bufs=1, space="PSUM"))
    ptp = ctx.enter_context(tc.tile_pool(name="ptp", bufs=7))
    smp = ctx.enter_context(tc.tile_pool(name="smp", bufs=4))
    obp = ctx.enter_context(tc.tile_pool(name="obp", bufs=3))

    # ---- resident inputs; DMA order gates the first matmuls on ~1.5MB
    wq = consts.tile([P, T, KC, P], BF16)
    wk = consts.tile([P, T, KC, P], BF16)
    wv = consts.tile([P, KC, HG * HD], BF16)
    wo = consts.tile([P, T, D], BF16)
    xt = consts.tile([P, NA, KC, QB], BF16)   # x^T: [p, m, c, l-within-m]
    # the first matmul only needs wk(t=0) + x(m=0): ~1.25MB gates the PE
    nc.sync.dma_start(out=wk[:, 0], in_=wk_d[0])
    nc.sync.dma_start(out=xt[:, 0], in_=xt_d[:, 0])
    nc.sync.dma_start(out=wq[:, 0], in_=wq_d[0])
    nc.sync.dma_start(out=xt[:, 1], in_=xt_d[:, 1])
    nc.sync.dma_start(out=xt[:, 2], in_=xt_d[:, 2])
    nc.sync.dma_start(out=wk[:, 1], in_=wk_d[1])
    nc.sync.dma_start(out=wq[:, 1], in_=wq_d[1])
    nc.sync.dma_start(out=xt[:, 3], in_=xt_d[:, 3])
    nc.sync.dma_start(out=wv, in_=wv_d)
    nc.sync.dma_start(out=wo, in_=wo_d)

    # ---- intermediates
    qt = consts.tile([P, T, L], BF16)         # Q^T: [d-of-pair, t, m]
    kt = consts.tile([P, T, L], BF16)
    vsb = consts.tile([P, NJ, HG, HD + 1], BF16)  # [key-in-blk, jb, h, d | ones]
    yt = consts.tile([P, T, L], BF16)         # normalized Y^T
    nc.vector.memset(vsb[:, :, :, HD:HD + 1], 1.0)

    cp_rot = [0]

    def copy_out(dst, src):
        # alternate PSUM->SBUF copies between ACT and DVE
        if cp_rot[0] % 2 == 0:
            nc.scalar.copy(out=dst, in_=src)
        else:
            nc.vector.tensor_copy(out=dst, in_=src)
        cp_rot[0] += 1

    # ---- projections (bf16, contraction over D in 8 chunks of 128)
    for t in range(T):
        for m in range(NA):
            msl = slice(m * QB, (m + 1) * QB)
            pk = pj.tile([P, QB], F32, tag="pj")
            for c in range(KC):
                nc.tensor.matmul(pk, lhsT=wk[:, t, c, :],
                                 rhs=xt[:, m, c, :], start=(c == 0), stop=(c == KC - 1))
            copy_out(kt[:, t, msl], pk)
            pq = pj.tile([P, QB], F32, tag="pj")
            for c in range(KC):
                nc.tensor.matmul(pq, lhsT=wq[:, t, c, :],
                                 rhs=xt[:, m, c, :], start=(c == 0), stop=(c == KC - 1))
            copy_out(qt[:, t, msl], pq)
    for jb in range(NJ):
        m, q = jb // 4, jb % 4
        pv = pj.tile([P, HG * HD], F32, tag="pj")
        for c in range(KC):
            nc.tensor.matmul(pv, lhsT=xt[:, m, c, q * P:(q + 1) * P],
                             rhs=wv[:, c, :], start=(c == 0), stop=(c == KC - 1))
        copy_out(vsb[:, jb, :, 0:HD], pv.rearrange("p (h d) -> p h d", h=HG))

    bal = _ExpBalancer()
    pending = []   # deferred normalization state: dict(a, t, drow, ysb, den)

    def norm_recip_bcast(pn):
        # step A (emitted at the START of the next attention block, so the
        # gpsimd broadcast clears the queue before that block's zeros):
        rec = smp.tile([1, 2, QB], F32, tag="rec")
        nc.vector.reciprocal_approx_fast(
            out=rec.rearrange("p a b -> p (a b)"),
            in_=pn["drow"].rearrange("p a b -> p (a b)"))
        den = smp.tile([64, 2, QB], F32, tag="den")
        nc.gpsimd.partition_broadcast(den, rec)
        pn["den"] = den

    def norm_muls(pn):
        # step B (emitted at the END of the next attention block: by then
        # the broadcast is long done, so these never block the DVE queue)
        a, t = pn["a"], pn["t"]
        for u in range(2):
            nc.vector.tensor_mul(yt[64 * u:64 * u + 64, t, a * QB:(a + 1) * QB],
                                 pn["ysb"][:, u, :], pn["den"][:, u, :])

    LOOK = 5   # AV(j) is emitted after scores(j+LOOK): the PE (strict FIFO)
               # executes 3 score-pairs' worth of work while exp(j) completes,
               # so AV(j) never stalls the PE on the exp latency.

    def attn(a, t, last):
        nj = 4 * a + 4
        if pending:
            norm_recip_bcast(pending[-1])
        psys = py.tile([65, 2, QB], F32, tag="py", name=f"psy{a}{t}")

        def emit_av(j, pt, off):
            for u in range(2):
                nc.tensor.matmul(
                    psys[:, u, off:QB],
                    lhsT=vsb[:, j, 2 * t + u, :],
                    rhs=pt[:, u, off:QB],
                    start=(j == 0), stop=(j == nj - 1),
                )

        def tail_chain(mi):
            # last block only: cols [128*mi, 128*(mi+1)) of psys are final
            # once AV(j=nj-4+mi) has run, so normalize them m-granularly
            # while the remaining AVs still stream on the PE
            msl = slice(mi * P, (mi + 1) * P)
            drow = smp.tile([1, 2, P], F32, tag="drowm")
            nc.vector.tensor_copy(out=drow, in_=psys[64:65, :, msl])
            rec = smp.tile([1, 2, P], F32, tag="recm")
            nc.vector.reciprocal_approx_fast(
                out=rec.rearrange("p a b -> p (a b)"),
                in_=drow.rearrange("p a b -> p (a b)"))
            den = smp.tile([64, 2, P], F32, tag="denm")
            nc.gpsimd.partition_broadcast(den, rec)
            for u in range(2):
                nc.vector.tensor_mul(
                    yt[64 * u:64 * u + 64, t,
                       a * QB + mi * P:a * QB + (mi + 1) * P],
                    psys[0:64, u, msl], den[:, u, :])

        stash = []
        for j in range(nj):
            r = j - 4 * a          # >= 0 on diagonal blocks
            off = 128 * r if r >= 0 else 0
            pss = ps.tile([P, 2, QB], F32, tag="ps")
            for u in range(2):
                hp = slice(64 * u, 64 * u + 64)
                nc.tensor.matmul(
                    pss[:, u, off:QB],
                    lhsT=kt[hp, t, j * P:(j + 1) * P],
                    rhs=qt[hp, t, a * QB + off:(a + 1) * QB],
                    start=True, stop=True,
                )
            pt = ptp.tile([P, 2, QB], BF16, tag="pt")
            elems = 2 * (QB - off)
            if bal.pick(elems) == "act":
                nc.scalar.activation(pt[:, :, off:QB], pss[:, :, off:QB],
                                     AF.Exp, scale=float(SCALE))
            else:
                nc.vector.tensor_scalar(
                    out=pt[:, :, off:QB].bitcast(I16), in0=pss[:, :, off:QB],
                    scalar1=float(SCALE * EXP_K), scalar2=EXP_B,
                    op0=mybir.AluOpType.mult, op1=mybir.AluOpType.add)
            if r >= 0:
                # zero the causal upper triangle of the diagonal strip:
                # keep iff (query-within-strip - key) >= 0
                nc.gpsimd.affine_select(
                    out=pt[:, :, off:off + P], in_=pt[:, :, off:off + P],
                    compare_op=mybir.AluOpType.is_ge, fill=0.0,
                    base=0, pattern=[[0, 2], [1, P]], channel_multiplier=-1)
            stash.append((j, pt, off))
            if len(stash) > LOOK:
                emit_av(*stash.pop(0))
        if last:
            if pending:
                norm_muls(pending.pop(0))
            for s in stash:
                emit_av(*s)
            for mi in range(4):
                tail_chain(mi)
                outproj_m(4 * a + mi)
            return
        for s in stash:
            emit_av(*s)
        if not last:
            # free psys ASAP: its only readers are these two copies
            ysb = smp.tile([64, 2, QB], F32, tag="ysb")
            nc.scalar.copy(out=ysb, in_=psys[0:64])
            drow = smp.tile([1, 2, QB], F32, tag="drow")
            nc.vector.tensor_copy(out=drow.rearrange("p a b -> p (a b)"),
                                  in_=psys[64:65].rearrange("p a b -> p (a b)"))
            if pending:
                norm_muls(pending.pop(0))
            pending.append({"a": a, "t": t, "drow": drow, "ysb": ysb})

    def outproj_m(m):
        for db in range(2):
            dsl = slice(db * QB, (db + 1) * QB)
            pso = pj.tile([P, QB], F32, tag="pj")
            for t in range(T):
                nc.tensor.matmul(
                    pso,
                    lhsT=yt[:, t, m * P:(m + 1) * P],
                    rhs=wo[:, t, dsl],
                    start=(t == 0), stop=(t == T - 1),
                )
            ob = obp.tile([P, QB], BF16, tag="ob")
            copy_out(ob, pso)
            nc.sync.dma_start(out=out_d[m, :, db], in_=ob)

    def outproj(a):
        for mi in range(4):
            outproj_m(4 * a + mi)

    # software-pipelined emission: outproj(a) after attn(a+1, t=0)
    attn(0, 0, False)
    attn(0, 1, False)
    attn(1, 0, False)
    outproj(0)
    attn(1, 1, False)
    attn(2, 0, False)
    outproj(1)
    attn(2, 1, False)
    attn(3, 0, False)
    outproj(2)
    attn(3, 1, True)


_NC_CACHE = None


def _build_nc():
    global _NC_CACHE
    if _NC_CACHE is not None:
        return _NC_CACHE
    nc = bacc.Bacc("TRN2", target_bir_lowering=False, debug=False,
                   enable_asserts=False)
    xt_d = nc.dram_tensor("xt", [P, NA, KC, QB], BF16, kind="ExternalInput")
    wq_d = nc.dram_tensor("wq", [T, P, KC, P], BF16, kind="ExternalInput")
    wk_d = nc.dram_tensor("wk", [T, P, KC, P], BF16, kind="ExternalInput")
    wv_d = nc.dram_tensor("wv", [P, KC, HG * HD], BF16, kind="ExternalInput")
    wo_d = nc.dram_tensor("wo", [P, T, D], BF16, kind="ExternalInput")
    out_d = nc.dram_tensor("out", [NJ, P, 2, QB], BF16, kind="ExternalOutput")
    with tile.TileContext(nc) as tc, ExitStack() as ctx:
        _body(ctx, tc, xt_d.ap(), wq_d.ap(), wk_d.ap(), wv_d.ap(), wo_d.ap(),
              out_d.ap())
    nc.compile()
    _NC_CACHE = nc
    return nc


def _shard_inputs(x_bld, Wq, Wk, Wv, Wo):
    x_bld = np.asarray(x_bld, dtype=np.float32)
    Wq = np.asarray(Wq, dtype=np.float32)
    Wk = np.asarray(Wk, dtype=np.float32)
    Wv = np.asarray(Wv, dtype=np.float32)
    Wo = np.asarray(Wo, dtype=np.float32)
    bf = ml_dtypes.bfloat16

    def wlay(Wm):       # [1024, 256] -> [128, 8, 256] (p, c, n)
        return np.ascontiguousarray(
            Wm.reshape(KC, P, HG * HD).transpose(1, 0, 2).astype(bf))

    def wlay_t(Wm):     # [1024, 256] -> [2, 128, 8, 128] (t, p, c, n)
        return np.ascontiguousarray(
            Wm.reshape(KC, P, T, P).transpose(2, 1, 0, 3).astype(bf))

    in_maps = []
    for c in range(NCORES):
        b, g = divmod(c, NG)
        hsl = slice(g * HG, (g + 1) * HG)
        # x^T [1024, 2048] -> [128, 4, 8, 512] (p, m, c, l)
        xT = x_bld[b].T.reshape(KC, P, NA, QB)
        xl = np.ascontiguousarray(xT.transpose(1, 2, 0, 3).astype(bf))
        # Wo [4, 64, 1024] -> [256, 1024] -> [128, 2, 1024] (p, t, d)
        woR = Wo[hsl].reshape(T, P, D)
        wol = np.ascontiguousarray(woR.transpose(1, 0, 2).astype(bf))
        in_maps.append({
            "xt": xl,
            "wq": wlay_t(Wq[:, hsl, :].reshape(D, HG * HD)),
            "wk": wlay_t(Wk[:, hsl, :].reshape(D, HG * HD)),
            "wv": wlay(Wv[:, hsl, :].reshape(D, HG * HD)),
            "wo": wol,
        })
    return in_maps


def _combine(outs):
    y = np.zeros((B, L, D), dtype=np.float32)
    for c in range(NCORES):
        y[c // NG] += np.asarray(outs[c], dtype=np.float32).reshape(L, D)
    return y


LAST_RESULT = None


def kernel(x_bld, Wq, Wk, Wv, Wo):
    global LAST_RESULT
    from concourse.bass_utils import run_bass_kernel_spmd
    nc = _build_nc()
    in_maps = _shard_inputs(x_bld, Wq, Wk, Wv, Wo)
    res = run_bass_kernel_spmd(nc, in_maps, core_ids=list(range(NCORES)))
    LAST_RESULT = res
    return _combine([res.results[c]["out"] for c in range(NCORES)])


# revision 25
# speedup vs baseline: 1.4391x; 1.0021x over previous
"""Causal self-attention kernel for Trainium2, distributed over 8 NeuronCores.

Problem (full): x[2, 2048, 1024], Wq/Wk/Wv[1024, 16, 64], Wo[16, 64, 1024]
  q/k/v = einsum('bld,dhk->blhk'); scores = q k^T / sqrt(64), causal mask,
  softmax; y = attn @ v; out = einsum('blhk,hkd->bld').

Sharding: core c in 0..7 -> batch b = c // 4, head-group g = c % 4
  (heads [4g, 4g+4)).  Each core computes its batch's partial output
  projection over its 4 heads; the host sums the 4 head-group partials
  per batch (the "all-reduce" of the output projection done host-side
  during unsharding).

Per-core design (bf16 matmuls, f32 PSUM accumulation):
  - All dram tensors are PRE-LAID-OUT host-side in on-chip order so every
    DMA is contiguous per partition (large packets), and the load order
    interleaves weights with x slices so the first projection matmul is
    gated on ~1.5MB, not ~5.5MB.
  - Q^T, K^T computed as [128(d of head-pair), 2, 2048]; scores for the
    two heads of a pair are issued to disjoint PE row groups (K=64,
    base partitions 0/64) so they run CONCURRENTLY.
  - softmax without max-subtraction (scores are O(10): exp is safe).
    The exp work is split across BOTH the Scalar engine (table exp) and
    the Vector engine (exp2 bit-trick: y = round(s*128*log2e + magic) as
    int16, bitcast to bf16 -- |rel err| <= 3.3%, washes out in softmax
    normalization + AV averaging), greedily balanced at build time.
  - causal mask: no additive -inf pass; instead gpsimd affine_select
    zeroes the upper triangle of the exp'd diagonal strip in SBUF
    (gpsimd is otherwise idle; DVE/ACT keep their cycles for exp).
  - denominator free via ones-column appended to V: AV matmul produces
    [y(64 rows); rowsum(1 row)] per head into one [65,2,512] PSUM tile.
  - normalization: psys is copied to SBUF immediately (frees PSUM for the
    next (a,t) AV), then reciprocal (DVE) + partition_broadcast (gpsimd)
    + multiply (DVE) produce normalized Y^T in bf16 -- exactly the lhsT
    layout the output projection needs.
  - emission order is software-pipelined: outproj(a) is emitted after
    attention(a+1, t=0), so the PE always has queued matmuls while the
    normalization chain of block a completes (keeps HAM un-throttled).
"""

import sys

sys.path.insert(0, "/opt/trn_rl_repo")

import os

import ml_dtypes
import numpy as np
from contextlib import ExitStack

_NO_DVE_EXP = os.environ.get("K_NO_DVE_EXP", "0") == "1"
_NO_TRI = os.environ.get("K_NO_TRI", "0") == "1"

import concourse.bass as bass
import concourse.mybir as mybir
import concourse.tile as tile
from concourse import bacc

F32 = mybir.dt.float32
BF16 = mybir.dt.bfloat16
I16 = mybir.dt.int16
AF = mybir.ActivationFunctionType

B, L, D, H, HD = 2, 2048, 1024, 16, 64
NCORES = 8
HG = 4              # heads per core
NG = H // HG        # 4 head-groups
T = HG // 2         # 2 head-pairs per core
P = 128
KC = D // P         # 8 contraction chunks for the projections
QB = 512            # query-range block (moving free dim)
NA = L // QB        # 4 query ranges
NJ = L // P         # 16 key blocks
SCALE = 1.0 / np.sqrt(HD)

# exp2 bit-trick constants (bf16 exponent starts at bit 7)
LOG2E = float(np.log2(np.e))
EXP_K = float(128.0 * LOG2E)          # multiplied by SCALE at the call site
EXP_B = float(127 * 128 - 5.1)        # minimax offset for truncation


class _ExpBalancer:
    """Greedy build-time assignment of exp blocks to ACT vs DVE."""

    def __init__(self):
        self.act_ns = 0.0
        self.dve_ns = 0.0

    def pick(self, elems):
        act_cost = (elems + 352) / 1.2
        dve_cost = (elems + 271) / 0.96
        if self.act_ns + act_cost <= self.dve_ns + dve_cost:
            self.act_ns += act_cost
            return "act"
        self.dve_ns += dve_cost
        return "dve"

    def charge(self, act_elems, dve_elems):
        self.act_ns += (act_elems + 352) / 1.2
        self.dve_ns += (dve_elems + 271) / 0.96


def _body(ctx: ExitStack, tc: tile.TileContext, xt_d, wq_d, wk_d, wv_d, wo_d, out_d):
    nc = tc.nc

    consts = ctx.enter_context(tc.tile_pool(name="consts", bufs=1))
    pj = ctx.enter_context(tc.tile_pool(name="pj", bufs=2, space="PSUM"))
    ps = ctx.enter_context(tc.tile_pool(name="ps", bufs=2, space="PSUM"))
    py = ctx.enter_context(tc.tile_pool(name="py", bufs=1, space="PSUM"))
    ptp = ctx.enter_context(tc.tile_pool(name="ptp", bufs=8))
    smp = ctx.enter_context(tc.tile_pool(name="smp", bufs=4))
    obp = ctx.enter_context(tc.tile_pool(name="obp", bufs=3))

    # ---- resident inputs; DMA order gates the first matmuls on ~1.5MB
    wq = consts.tile([P, T, KC, P], BF16)
    wk = consts.tile([P, T, KC, P], BF16)
    wv = consts.tile([P, KC, HG * HD], BF16)
    wo = consts.tile([P, T, D], BF16)
    xt = consts.tile([P, NA, KC, QB], BF16)   # x^T: [p, m, c, l-within-m]
    # the first matmul only needs wk(t=0) + x(m=0): ~1.25MB gates the PE
    nc.sync.dma_start(out=wk[:, 0], in_=wk_d[0])
    nc.sync.dma_start(out=xt[:, 0], in_=xt_d[:, 0])
    nc.sync.dma_start(out=wq[:, 0], in_=wq_d[0])
    nc.sync.dma_start(out=xt[:, 1], in_=xt_d[:, 1])
    nc.sync.dma_start(out=xt[:, 2], in_=xt_d[:, 2])
    nc.sync.dma_start(out=wk[:, 1], in_=wk_d[1])
    nc.sync.dma_start(out=wq[:, 1], in_=wq_d[1])
    nc.sync.dma_start(out=xt[:, 3], in_=xt_d[:, 3])
    nc.sync.dma_start(out=wv, in_=wv_d)
    nc.sync.dma_start(out=wo, in_=wo_d)

    # ---- intermediates
    qt = consts.tile([P, T, L], BF16)         # Q^T: [d-of-pair, t, m]
    kt = consts.tile([P, T, L], BF16)
    vsb = consts.tile([P, NJ, HG, HD + 1], BF16)  # [key-in-blk, jb, h, d | ones]
    yt = consts.tile([P, T, L], BF16)         # normalized Y^T
    nc.vector.memset(vsb[:, :, :, HD:HD + 1], 1.0)

    cp_rot = [0]

    def copy_out(dst, src):
        # alternate PSUM->SBUF copies between ACT and DVE
        if cp_rot[0] % 2 == 0:
            nc.scalar.copy(out=dst, in_=src)
        else:
            nc.vector.tensor_copy(out=dst, in_=src)
        cp_rot[0] += 1

    # ---- projections (bf16, contraction over D in 8 chunks of 128)
    for t in range(T):
        for m in range(NA):
            msl = slice(m * QB, (m + 1) * QB)
            pk = pj.tile([P, QB], F32, tag="pj")
            for c in range(KC):
                nc.tensor.matmul(pk, lhsT=wk[:, t, c, :],
                                 rhs=xt[:, m, c, :], start=(c == 0), stop=(c == KC - 1))
            copy_out(kt[:, t, msl], pk)
            pq = pj.tile([P, QB], F32, tag="pj")
            for c in range(KC):
                nc.tensor.matmul(pq, lhsT=wq[:, t, c, :],
                                 rhs=xt[:, m, c, :], start=(c == 0), stop=(c == KC - 1))
            copy_out(qt[:, t, msl], pq)
    for jb in range(NJ):
        m, q = jb // 4, jb % 4
        pv = pj.tile([P, HG * HD], F32, tag="pj")
        for c in range(KC):
            nc.tensor.matmul(pv, lhsT=xt[:, m, c, q * P:(q + 1) * P],
                             rhs=wv[:, c, :], start=(c == 0), stop=(c == KC - 1))
        copy_out(vsb[:, jb, :, 0:HD], pv.rearrange("p (h d) -> p h d", h=HG))

    bal = _ExpBalancer()
    pending = []   # deferred normalization state: dict(a, t, drow, ysb, den)

    def norm_recip_bcast(pn):
        # step A (emitted at the START of the next attention block, so the
        # gpsimd broadcast clears the queue before that block's zeros):
        rec = smp.tile([1, 2, QB], F32, tag="rec")
        nc.vector.reciprocal_approx_fast(
            out=rec.rearrange("p a b -> p (a b)"),
            in_=pn["drow"].rearrange("p a b -> p (a b)"))
        den = smp.tile([64, 2, QB], F32, tag="den")
        nc.gpsimd.partition_broadcast(den, rec)
        pn["den"] = den

    def norm_muls(pn):
        # step B (emitted at the END of the next attention block: by then
        # the broadcast is long done, so these never block the DVE queue)
        a, t = pn["a"], pn["t"]
        for u in range(2):
            nc.vector.tensor_mul(yt[64 * u:64 * u + 64, t, a * QB:(a + 1) * QB],
                                 pn["ysb"][:, u, :], pn["den"][:, u, :])

    LOOK = 6   # AV(j) is emitted after scores(j+LOOK): the PE (strict FIFO)
               # executes 3 score-pairs' worth of work while exp(j) completes,
               # so AV(j) never stalls the PE on the exp latency.

    def attn(a, t, last):
        nj = 4 * a + 4
        if pending:
            norm_recip_bcast(pending[-1])
        psys = py.tile([65, 2, QB], F32, tag="py", name=f"psy{a}{t}")

        def emit_av(j, pt, off):
            for u in range(2):
                nc.tensor.matmul(
                    psys[:, u, off:QB],
                    lhsT=vsb[:, j, 2 * t + u, :],
                    rhs=pt[:, u, off:QB],
                    start=(j == 0), stop=(j == nj - 1),
                )

        def tail_chain(mi):
            # last block only: cols [128*mi, 128*(mi+1)) of psys are final
            # once AV(j=nj-4+mi) has run, so normalize them m-granularly
            # while the remaining AVs still stream on the PE
            msl = slice(mi * P, (mi + 1) * P)
            drow = smp.tile([1, 2, P], F32, tag="drowm")
            nc.vector.tensor_copy(out=drow, in_=psys[64:65, :, msl])
            rec = smp.tile([1, 2, P], F32, tag="recm")
            nc.vector.reciprocal_approx_fast(
                out=rec.rearrange("p a b -> p (a b)"),
                in_=drow.rearrange("p a b -> p (a b)"))
            den = smp.tile([64, 2, P], F32, tag="denm")
            nc.gpsimd.partition_broadcast(den, rec)
            for u in range(2):
                nc.vector.tensor_mul(
                    yt[64 * u:64 * u + 64, t,
                       a * QB + mi * P:a * QB + (mi + 1) * P],
                    psys[0:64, u, msl], den[:, u, :])

        stash = []
        for j in range(nj):
            r = j - 4 * a          # >= 0 on diagonal blocks
            off = 128 * r if r >= 0 else 0
            pss = ps.tile([P, 2, QB], F32, tag="ps")
            for u in range(2):
                hp = slice(64 * u, 64 * u + 64)
                nc.tensor.matmul(
                    pss[:, u, off:QB],
                    lhsT=kt[hp, t, j * P:(j + 1) * P],
                    rhs=qt[hp, t, a * QB + off:(a + 1) * QB],
                    start=True, stop=True,
                )
            pt = ptp.tile([P, 2, QB], BF16, tag="pt")
            elems = 2 * (QB - off)
            if bal.pick(elems) == "act":
                nc.scalar.activation(pt[:, :, off:QB], pss[:, :, off:QB],
                                     AF.Exp, scale=float(SCALE))
            else:
                nc.vector.tensor_scalar(
                    out=pt[:, :, off:QB].bitcast(I16), in0=pss[:, :, off:QB],
                    scalar1=float(SCALE * EXP_K), scalar2=EXP_B,
                    op0=mybir.AluOpType.mult, op1=mybir.AluOpType.add)
            if r >= 0:
                # zero the causal upper triangle of the diagonal strip:
                # keep iff (query-within-strip - key) >= 0
                nc.gpsimd.affine_select(
                    out=pt[:, :, off:off + P], in_=pt[:, :, off:off + P],
                    compare_op=mybir.AluOpType.is_ge, fill=0.0,
                    base=0, pattern=[[0, 2], [1, P]], channel_multiplier=-1)
            stash.append((j, pt, off))
            if len(stash) > LOOK:
                emit_av(*stash.pop(0))
        if last:
            if pending:
                norm_muls(pending.pop(0))
            for s in stash:
                emit_av(*s)
            for mi in range(4):
                tail_chain(mi)
                outproj_m(4 * a + mi)
            return
        for s in stash:
            emit_av(*s)
        if not last:
            # free psys ASAP: its only readers are these two copies
            ysb = smp.tile([64, 2, QB], F32, tag="ysb")
            nc.scalar.copy(out=ysb, in_=psys[0:64])
            drow = smp.tile([1, 2, QB], F32, tag="drow")
            nc.vector.tensor_copy(out=drow.rearrange("p a b -> p (a b)"),
                                  in_=psys[64:65].rearrange("p a b -> p (a b)"))
            if pending:
                norm_muls(pending.pop(0))
            pending.append({"a": a, "t": t, "drow": drow, "ysb": ysb})

    def outproj_m(m):
        for db in range(2):
            dsl = slice(db * QB, (db + 1) * QB)
            pso = pj.tile([P, QB], F32, tag="pj")
            for t in range(T):
                nc.tensor.matmul(
                    pso,
                    lhsT=yt[:, t, m * P:(m + 1) * P],
                    rhs=wo[:, t, dsl],
                    start=(t == 0), stop=(t == T - 1),
                )
            ob = obp.tile([P, QB], BF16, tag="ob")
            copy_out(ob, pso)
            nc.sync.dma_start(out=out_d[m, :, db], in_=ob)

    def outproj(a):
        for mi in range(4):
            outproj_m(4 * a + mi)

    # software-pipelined emission: outproj(a) after attn(a+1, t=0)
    attn(0, 0, False)
    attn(0, 1, False)
    attn(1, 0, False)
    outproj(0)
    attn(1, 1, False)
    attn(2, 0, False)
    outproj(1)
    attn(2, 1, False)
    attn(3, 0, False)
    outproj(2)
    attn(3, 1, True)


_NC_CACHE = None


def _build_nc():
    global _NC_CACHE
    if _NC_CACHE is not None:
        return _NC_CACHE
    nc = bacc.Bacc("TRN2", target_bir_lowering=False, debug=False,
                   enable_asserts=False)
    xt_d = nc.dram_tensor("xt", [P, NA, KC, QB], BF16, kind="ExternalInput")
    wq_d = nc.dram_tensor("wq", [T, P, KC, P], BF16, kind="ExternalInput")
    wk_d = nc.dram_tensor("wk", [T, P, KC, P], BF16, kind="ExternalInput")
    wv_d = nc.dram_tensor("wv", [P, KC, HG * HD], BF16, kind="ExternalInput")
    wo_d = nc.dram_tensor("wo", [P, T, D], BF16, kind="ExternalInput")
    out_d = nc.dram_tensor("out", [NJ, P, 2, QB], BF16, kind="ExternalOutput")
    with tile.TileContext(nc) as tc, ExitStack() as ctx:
        _body(ctx, tc, xt_d.ap(), wq_d.ap(), wk_d.ap(), wv_d.ap(), wo_d.ap(),
              out_d.ap())
    nc.compile()
    _NC_CACHE = nc
    return nc


def _shard_inputs(x_bld, Wq, Wk, Wv, Wo):
    x_bld = np.asarray(x_bld, dtype=np.float32)
    Wq = np.asarray(Wq, dtype=np.float32)
    Wk = np.asarray(Wk, dtype=np.float32)
    Wv = np.asarray(Wv, dtype=np.float32)
    Wo = np.asarray(Wo, dtype=np.float32)
    bf = ml_dtypes.bfloat16

    def wlay(Wm):       # [1024, 256] -> [128, 8, 256] (p, c, n)
        return np.ascontiguousarray(
            Wm.reshape(KC, P, HG * HD).transpose(1, 0, 2).astype(bf))

    def wlay_t(Wm):     # [1024, 256] -> [2, 128, 8, 128] (t, p, c, n)
        return np.ascontiguousarray(
            Wm.reshape(KC, P, T, P).transpose(2, 1, 0, 3).astype(bf))

    in_maps = []
    for c in range(NCORES):
        b, g = divmod(c, NG)
        hsl = slice(g * HG, (g + 1) * HG)
        # x^T [1024, 2048] -> [128, 4, 8, 512] (p, m, c, l)
        xT = x_bld[b].T.reshape(KC, P, NA, QB)
        xl = np.ascontiguousarray(xT.transpose(1, 2, 0, 3).astype(bf))
        # Wo [4, 64, 1024] -> [256, 1024] -> [128, 2, 1024] (p, t, d)
        woR = Wo[hsl].reshape(T, P, D)
        wol = np.ascontiguousarray(woR.transpose(1, 0, 2).astype(bf))
        in_maps.append({
            "xt": xl,
            "wq": wlay_t(Wq[:, hsl, :].reshape(D, HG * HD)),
            "wk": wlay_t(Wk[:, hsl, :].reshape(D, HG * HD)),
            "wv": wlay(Wv[:, hsl, :].reshape(D, HG * HD)),
            "wo": wol,
        })
    return in_maps


def _combine(outs):
    y = np.zeros((B, L, D), dtype=np.float32)
    for c in range(NCORES):
        y[c // NG] += np.asarray(outs[c], dtype=np.float32).reshape(L, D)
    return y


LAST_RESULT = None


def kernel(x_bld, Wq, Wk, Wv, Wo):
    global LAST_RESULT
    from concourse.bass_utils import run_bass_kernel_spmd
    nc = _build_nc()
    in_maps = _shard_inputs(x_bld, Wq, Wk, Wv, Wo)
    res = run_bass_kernel_spmd(nc, in_maps, core_ids=list(range(NCORES)))
    LAST_RESULT = res
    return _combine([res.results[c]["out"] for c in range(NCORES)])
